# revision 5
# baseline (speedup 1.0000x reference)
"""ARX (order-16 IIR + order-16 FIR) over a 2^20-step sequence on 8 TRN2 cores.

Method: the stable AR(16) recurrence is converted to an equivalent truncated
FIR filter.  With the problem's coefficient scaling (sum|A| <= 0.9) the AR
impulse response h decays geometrically; 256 combined taps w = conv(h, B)
put the truncation error ~1e-12, far below fp32 noise.

    y[p] = sum_{m=0}^{255} w[m] * z[p-m],   z[q] = u[q+15]

The convolution runs as block-Toeplitz matmuls on the TensorEngine: the
sequence is laid out interleaved (X[t, c] = z[128*c + t]) so the contraction
dim (fine time shift) sits in partitions, and three 128x128 Toeplitz weight
matrices (lower-tri / dense / upper-tri slices of w) accumulate into PSUM
over shifted column windows.  Outputs are sharded 8 x 131072 across cores
(data-parallel over the sequence with a 256-sample halo - no collectives).

The first 256 outputs depend on the zero initial state (the FIR form assumes
an infinite past), so they are computed exactly on the host (256-step
recurrence in float64) and overwrite the device result - 0.02% of the output.
"""

import numpy as np

import concourse.bass as bass
import concourse.mybir as mybir
from concourse.bass_utils import run_bass_kernel_spmd

NCORES = 8
N = 1 << 20                # outputs
PER = N // NCORES          # 131072 outputs per core
QCOLS = PER // 128         # 1024 interleaved columns per core
FREE = 512                 # PSUM bank = 512 fp32
GROUPS = QCOLS // FREE     # 2

# Diagnostics for the local test harness (not used by grading).
LAST_RESULTS = None


def _fir_taps(a64: np.ndarray, b64: np.ndarray):
    """Truncated impulse response of the full ARX transfer function.

    Returns (w, S): w has 128*S - 128 + 128 = 128*(S-1)+128 taps arranged so
    S Toeplitz blocks cover it; taps chosen so the discarded tail is < 1e-9.
    """
    cap = 4096
    h = np.zeros(cap, dtype=np.float64)
    h[0] = 1.0
    for m in range(1, cap):
        k = min(16, m)
        h[m] = a64[:k] @ h[m - k:m][::-1]
    absh = np.abs(h)
    tail = np.cumsum(absh[::-1])[::-1]
    # S Toeplitz blocks guarantee every output sees taps [0, 128*(S-1)];
    # taps beyond that are applied only for outputs with large i (harmless).
    S = 3
    while 128 * S < cap - 16 and tail[128 * (S - 1)] > 1e-9:
        S += 1
    M = 128 * S
    w = np.convolve(h[:M - 15], b64)  # length M
    return w, S


def _toeplitz_weights(w32: np.ndarray, S: int) -> np.ndarray:
    """[128, S*128] fp32: columns [128s:128s+128] hold W_s[t,i]=w[i-t+128s]."""
    M = len(w32)
    t = np.arange(128)[:, None]
    i = np.arange(128)[None, :]
    Wmat = np.zeros((128, S * 128), dtype=np.float32)
    for s in range(S):
        m = i - t + 128 * s
        valid = (m >= 0) & (m < M)
        Wmat[:, 128 * s:128 * s + 128] = np.where(valid, w32[np.clip(m, 0, M - 1)], 0.0)
    return Wmat


def _build_nc(S: int) -> bass.Bass:
    xcols = QCOLS + S - 1
    f32 = mybir.dt.float32
    nc = bass.Bass()
    x_in = nc.declare_dram_parameter("x", [128, xcols], f32, isOutput=False)
    w_in = nc.declare_dram_parameter("w", [128, S * 128], f32, isOutput=False)
    y_out = nc.declare_dram_parameter("y", [128, QCOLS], f32, isOutput=True)

    xt = nc.alloc_sbuf_tensor("xt", [128, xcols], f32)
    wt = nc.alloc_sbuf_tensor("wt", [128, S * 128], f32)
    yt = nc.alloc_sbuf_tensor("yt", [128, QCOLS], f32)
    ps = [nc.alloc_psum_tensor(f"ps{g}", [128, FREE], f32) for g in range(GROUPS)]

    with nc.Block() as block, \
         nc.semaphore("dma_sem") as dma_sem, \
         nc.semaphore("mm_sem") as mm_sem, \
         nc.semaphore("cp_sem") as cp_sem:

        @block.sync
        def _(sync: bass.BassEngine):
            sync.dma_start(out=xt[:], in_=x_in[:]).then_inc(dma_sem, 16)
            sync.dma_start(out=wt[:], in_=w_in[:]).then_inc(dma_sem, 16)
            sync.wait_ge(cp_sem, GROUPS)
            sync.dma_start(out=y_out[:], in_=yt[:]).then_inc(dma_sem, 16)
            sync.wait_ge(dma_sem, 48)

        @block.tensor
        def _(tensor: bass.BassEngine):
            tensor.wait_ge(dma_sem, 32)
            for g in range(GROUPS):
                for s in range(S):
                    off = FREE * g + (S - 1) - s
                    mm = tensor.matmul(
                        ps[g][:],
                        wt[:, 128 * s:128 * s + 128],
                        xt[:, off:off + FREE],
                        start=(s == 0),
                        stop=(s == S - 1),
                    )
                mm.then_inc(mm_sem)

        @block.vector
        def _(vector: bass.BassEngine):
            for g in range(GROUPS):
                vector.wait_ge(mm_sem, g + 1)
                vector.tensor_copy(yt[:, FREE * g:FREE * g + FREE], ps[g][:]).then_inc(cp_sem)

    return nc


def _boundary_exact(u64, a64, b64, n):
    """First n outputs of the reference recurrence, float64."""
    y = np.zeros(n, dtype=np.float64)
    d = np.convolve(u64[:n + 16], b64)[15:15 + n]
    for k in range(n):
        k0 = max(0, k - 16)
        acc = d[k] + a64[:k - k0] @ y[k - 1:k0 - 1 if k0 > 0 else None:-1][:k - k0]
        y[k] = acc
    return y


def kernel(u, A_w, B_w):
    global LAST_RESULTS
    import os

    u = np.asarray(u, dtype=np.float32)
    a64 = np.asarray(A_w, dtype=np.float64).ravel()
    b64 = np.asarray(B_w, dtype=np.float64).ravel()

    w, S = _fir_taps(a64, b64)
    M = len(w)
    w32 = w.astype(np.float32)
    Wmat = _toeplitz_weights(w32, S)

    # padded, advanced input: zp[j] = z[j - M] with z[q] = u[q + 15]
    zpad = np.zeros(M + N, dtype=np.float32)
    zpad[M - 15:] = u[:N + 15]
    pad_cols = S - 1  # halo columns in front of each core's window
    xcols = QCOLS + pad_cols

    in_maps = []
    for core in range(NCORES):
        p0 = core * PER
        # Xz[t, c] = z[p0 + 128*(c - pad_cols) + t] = zpad[p0 + M - 128*pad_cols + 128*c + t]
        j0 = p0 + M - 128 * pad_cols
        seg = zpad[j0:j0 + 128 * xcols]
        Xz = np.ascontiguousarray(seg.reshape(xcols, 128).T)
        in_maps.append({"x": Xz, "w": Wmat})

    nc = _build_nc(S)
    trace = False
    if os.environ.get("KERNEL_TRACE"):
        try:
            import antenv.axon_hooks  # noqa: F401  (shim installed by test.py)
            trace = True
        except ImportError:
            pass
    res = run_bass_kernel_spmd(nc, in_maps, list(range(NCORES)), trace=trace)
    LAST_RESULTS = res

    y = np.empty(N, dtype=np.float32)
    for core in range(NCORES):
        Y = np.asarray(res.results[core]["y"])
        y[core * PER:(core + 1) * PER] = Y.T.reshape(-1)

    # exact initial-condition boundary (first M outputs)
    u64 = u.astype(np.float64)
    y[:M] = _boundary_exact(u64, a64, b64, M).astype(np.float32)
    return y


# revision 6
# speedup vs baseline: 1.2506x; 1.2506x over previous
"""ARX (order-16 IIR + order-16 FIR) over a 2^20-step sequence on 8 TRN2 cores.

Method: the stable AR(16) recurrence is converted to an equivalent truncated
FIR filter.  With the problem's coefficient scaling (sum|A| <= 0.9) the AR
impulse response h decays geometrically; 256 combined taps w = conv(h, B)
put the truncation error at the fp32 noise floor (~1e-7).

    y[p] = sum_{m} w[m] * z[p-m],   z[q] = u[q+15]

The convolution runs as block-Toeplitz matmuls on the TensorEngine: the
sequence is laid out interleaved (X[t, c] = z[128*c + t]) so the contraction
dim (fine time shift) sits in partitions, and two 128x128 Toeplitz weight
matrices (lower-triangular / dense slices of w) accumulate into PSUM over
shifted column windows.  Outputs are sharded 8 x 131072 across cores
(data-parallel over the sequence with a 256-sample halo - no collectives).
The device program is pipelined: chunked input DMA (sync engine), 4 PSUM
groups of 256 columns on the TensorEngine (with warmup matmuls during the
DMA window to lift the HAM clock gate), PSUM->SBUF copies on the Vector
engine, chunked output DMA on the Scalar engine.

The first 256 outputs depend on the zero initial state (the FIR form assumes
an infinite past), so they are computed exactly on the host (256-step
recurrence in float64) and overwrite the device result - 0.02% of the output.
"""

import os

import numpy as np

import concourse.bass as bass
import concourse.mybir as mybir
from concourse.bass_utils import run_bass_kernel_spmd

NCORES = 8
N = 1 << 20                # outputs
PER = N // NCORES          # 131072 outputs per core
QCOLS = PER // 128         # 1024 interleaved columns per core
GFREE = 256                # columns per PSUM group
NGROUPS = QCOLS // GFREE   # 4

# Diagnostics for the local test harness (not used by grading).
LAST_RESULTS = None


def _fir_taps(a64: np.ndarray, b64: np.ndarray):
    """Truncated impulse response of the full ARX transfer function.

    Returns (w, S): with S Toeplitz blocks every output is guaranteed taps
    [0, 128*(S-1)]; S chosen so the discarded tail is below fp32 noise.
    """
    cap = 4096
    h = np.zeros(cap, dtype=np.float64)
    h[0] = 1.0
    for m in range(1, cap):
        k = min(16, m)
        h[m] = a64[:k] @ h[m - k:m][::-1]
    absh = np.abs(h)
    tail = np.cumsum(absh[::-1])[::-1]
    S = 2
    while 128 * S < cap - 16 and tail[128 * (S - 1)] > 3e-7:
        S += 1
    M = 128 * S
    w = np.convolve(h[:M - 15], b64)  # length M
    return w, S


def _toeplitz_weights(w32: np.ndarray, S: int) -> np.ndarray:
    """[128, S*128] fp32: columns [128s:128s+128] hold W_s[t,i]=w[i-t+128s]."""
    M = len(w32)
    t = np.arange(128)[:, None]
    i = np.arange(128)[None, :]
    Wmat = np.zeros((128, S * 128), dtype=np.float32)
    for s in range(S):
        m = i - t + 128 * s
        valid = (m >= 0) & (m < M)
        Wmat[:, 128 * s:128 * s + 128] = np.where(valid, w32[np.clip(m, 0, M - 1)], 0.0)
    return Wmat


def _build_nc(S: int, mm_dtype: str, warmup: int) -> bass.Bass:
    xcols = QCOLS + S - 1
    f32 = mybir.dt.float32
    in_dt = mybir.dt.float32r if mm_dtype == "f32r" else f32
    nc = bass.Bass()
    x_in = nc.declare_dram_parameter("x", [128, xcols], in_dt, isOutput=False)
    w_in = nc.declare_dram_parameter("w", [128, S * 128], in_dt, isOutput=False)
    y_out = nc.declare_dram_parameter("y", [128, QCOLS], f32, isOutput=True)

    xt = nc.alloc_sbuf_tensor("xt", [128, xcols], in_dt)
    wt = nc.alloc_sbuf_tensor("wt", [128, S * 128], in_dt)
    yt = nc.alloc_sbuf_tensor("yt", [128, QCOLS], f32)
    # one full PSUM bank per group to guarantee bank separation
    ps = [nc.alloc_psum_tensor(f"ps{g}", [128, 512], f32) for g in range(NGROUPS)]
    # warmup scratch (uninitialized SBUF is fine - results are discarded)
    if warmup:
        wu_in = nc.alloc_sbuf_tensor("wu_in", [128, GFREE], f32)
        wu_w = nc.alloc_sbuf_tensor("wu_w", [128, 128], f32)
        wu_ps = nc.alloc_psum_tensor("wu_ps", [128, 512], f32)

    # x chunk 0 covers columns [0, 512+S-1), chunk 1 the rest
    xc0 = 512 + S - 1

    with nc.Block() as block, \
         nc.semaphore("dma_sem") as dma_sem, \
         nc.semaphore("mm_sem") as mm_sem, \
         nc.semaphore("cp_sem") as cp_sem, \
         nc.semaphore("out_sem") as out_sem:

        @block.sync
        def _(sync: bass.BassEngine):
            sync.dma_start(out=wt[:], in_=w_in[:]).then_inc(dma_sem, 16)
            sync.dma_start(out=xt[:, :xc0], in_=x_in[:, :xc0]).then_inc(dma_sem, 16)
            sync.dma_start(out=xt[:, xc0:], in_=x_in[:, xc0:]).then_inc(dma_sem, 16)
            sync.wait_ge(dma_sem, 48)

        @block.tensor
        def _(tensor: bass.BassEngine):
            for _ in range(warmup):
                tensor.matmul(wu_ps[:, :GFREE], wu_w[:], wu_in[:],
                              start=True, stop=True)
            tensor.wait_ge(dma_sem, 32)  # w + x chunk 0
            for g in range(NGROUPS):
                if g == 2:
                    tensor.wait_ge(dma_sem, 48)  # x chunk 1
                for s in range(S):
                    off = GFREE * g + (S - 1) - s
                    mm = tensor.matmul(
                        ps[g][:, :GFREE],
                        wt[:, 128 * s:128 * s + 128],
                        xt[:, off:off + GFREE],
                        start=(s == 0),
                        stop=(s == S - 1),
                    )
                mm.then_inc(mm_sem)

        @block.vector
        def _(vector: bass.BassEngine):
            for g in range(NGROUPS):
                vector.wait_ge(mm_sem, g + 1)
                vector.tensor_copy(
                    yt[:, GFREE * g:GFREE * (g + 1)], ps[g][:, :GFREE]
                ).then_inc(cp_sem)

        @block.scalar
        def _(scalar: bass.BassEngine):
            scalar.wait_ge(cp_sem, 2)
            scalar.dma_start(out=y_out[:, :512], in_=yt[:, :512]).then_inc(out_sem, 16)
            scalar.wait_ge(cp_sem, 4)
            scalar.dma_start(out=y_out[:, 512:], in_=yt[:, 512:]).then_inc(out_sem, 16)
            scalar.wait_ge(out_sem, 32)

    return nc


def _boundary_exact(u64, a64, b64, n):
    """First n outputs of the reference recurrence, float64."""
    y = np.zeros(n, dtype=np.float64)
    d = np.convolve(u64[:n + 16], b64)[15:15 + n]
    for k in range(n):
        acc = d[k]
        for j in range(min(16, k)):
            acc += a64[j] * y[k - 1 - j]
        y[k] = acc
    return y


def kernel(u, A_w, B_w):
    global LAST_RESULTS

    u = np.asarray(u, dtype=np.float32)
    a64 = np.asarray(A_w, dtype=np.float64).ravel()
    b64 = np.asarray(B_w, dtype=np.float64).ravel()

    w, S = _fir_taps(a64, b64)
    M = len(w)
    Wmat = _toeplitz_weights(w.astype(np.float32), S)

    # padded, advanced input: zp[j] = z[j - M] with z[q] = u[q + 15]
    zpad = np.zeros(M + N, dtype=np.float32)
    zpad[M - 15:] = u[:N + 15]
    pad_cols = S - 1
    xcols = QCOLS + pad_cols

    in_maps = []
    for core in range(NCORES):
        p0 = core * PER
        # Xz[t, c] = z[p0 + 128*(c - pad_cols) + t]
        j0 = p0 + M - 128 * pad_cols
        seg = zpad[j0:j0 + 128 * xcols]
        Xz = np.ascontiguousarray(seg.reshape(xcols, 128).T)
        in_maps.append({"x": Xz, "w": Wmat})

    mm_dtype = os.environ.get("KERNEL_MM_DTYPE", "fp32")
    warmup = int(os.environ.get("KERNEL_WARMUP", "4"))
    nc = _build_nc(S, mm_dtype, warmup)

    trace = False
    if os.environ.get("KERNEL_TRACE"):
        try:
            import antenv.axon_hooks  # noqa: F401  (shim installed by test.py)
            trace = True
        except ImportError:
            pass
    res = run_bass_kernel_spmd(nc, in_maps, list(range(NCORES)), trace=trace)
    LAST_RESULTS = res

    y = np.empty(N, dtype=np.float32)
    for core in range(NCORES):
        Y = np.asarray(res.results[core]["y"])
        y[core * PER:(core + 1) * PER] = Y.T.reshape(-1)

    # exact initial-condition boundary (first M outputs)
    y[:M] = _boundary_exact(u.astype(np.float64), a64, b64, M).astype(np.float32)
    return y
